# revision 20
# baseline (speedup 1.0000x reference)
"""Trainium2 Bass kernel for nn_EncoderRNN: 2-layer bidirectional GRU encoder.

Design (8 NeuronCores, one SPMD NEFF):
 - Feature-on-partition ("transposed") layout on device. Host does the cheap
   index gathers, per-core input layout transforms, and final reassembly.
 - Cores 0-3 run forward chains, 4-7 backward chains; all direction asymmetry
   (time order, weight halves, blend masks) is encoded in per-core input DATA
   so a single instruction stream serves all cores.
 - Per layer: xW = x @ Wih.T precomputed as a fat fp16 matmul spilled to local
   DRAM and streamed back per step; the recurrence streams Whh.T 128x128
   stationary tiles against the live h state (fp16 operands, fp32 PSUM).
 - One 8-rank AllGather per layer exchanges full y trajectories.
 - Classifier is vocab-sharded (1000 rows/core) over the [8000,512] projection.
"""
import sys
import numpy as np

try:
    import concourse.bass  # noqa: F401  (already on path in some environments)
except ImportError:
    sys.path.insert(0, '/opt/trn_rl_repo')

import concourse.bass as bass
import concourse.mybir as mybir
import concourse.tile as tile
from concourse import bacc
from concourse.bass_utils import run_bass_kernel_spmd

T, B, H, V, ED = 48, 128, 512, 8000, 32
IN0, K0 = 576, 5            # layer0 input width, padded to 640 = 5*128
G, NJ, KH = 1536, 12, 4
NC = 8
VS = V // NC                # 1000
TB = T * B                  # 6144
F16 = mybir.dt.float16
F32 = mybir.dt.float32

_CACHE = {}


def _build_program():
    nc = bacc.Bacc("TRN2", target_bir_lowering=False, debug=False,
                   enable_asserts=False, num_devices=NC)
    dt = nc.dram_tensor
    QB = 12 * B  # 1536 columns: one chain-quarter of positions
    jointT = dt("jointT", [128, K0, QB], F16, kind="ExternalInput")
    wih0T = dt("wih0T", [128, K0, G], F16, kind="ExternalInput")
    whh0T = dt("whh0T", [128, KH, G], F16, kind="ExternalInput")
    wih1To = dt("wih1To", [128, KH, G], F16, kind="ExternalInput")
    wih1Tx = dt("wih1Tx", [128, KH, G], F16, kind="ExternalInput")
    whh1T = dt("whh1T", [128, KH, G], F16, kind="ExternalInput")
    b0T = dt("b0T", [128, NJ], F32, kind="ExternalInput")
    b1T = dt("b1T", [128, NJ], F32, kind="ExternalInput")
    bhn0T = dt("bhn0T", [128, KH], F32, kind="ExternalInput")
    bhn1T = dt("bhn1T", [128, KH], F32, kind="ExternalInput")
    cw1T = dt("cw1T", [128, KH, H], F16, kind="ExternalInput")
    cb1T = dt("cb1T", [128, KH], F32, kind="ExternalInput")
    cw2T = dt("cw2T", [128, KH, VS], F16, kind="ExternalInput")
    cb2b = dt("cb2b", [128, VS], F32, kind="ExternalInput")
    mdir = dt("mdir", [128, 1], F32, kind="ExternalInput")   # 1.0 on fwd cores, 0.0 on bwd
    wq = dt("wq", [128, 4], F32, kind="ExternalInput")       # one-hot quarter select

    out_cls = dt("out_cls", [T, B, VS], F16, kind="ExternalOutput")
    out_oT = dt("out_oT", [128, KH, TB], F16, kind="ExternalOutput")
    out_hidA = dt("out_hidA", [128, KH, B], F16, kind="ExternalOutput")
    out_hidB = dt("out_hidB", [128, KH, B], F16, kind="ExternalOutput")

    with tile.TileContext(nc) as tc:
        with tc.tile_pool(name="pers", bufs=1) as pers, \
             tc.tile_pool(name="dram", bufs=1, space="DRAM") as dram:

            QB = 12 * B
            ag0_in = dram.tile([128, KH * TB], F16)
            ag0_out = dram.tile([NC * 128, KH * TB], F16)
            ag1_in = dram.tile([128, KH * TB], F16)
            ag1_out = dram.tile([NC * 128, KH * TB], F16)
            agx0_in = dram.tile([128, NJ * QB], F16)
            agx0_out = dram.tile([NC * 128, NJ * QB], F16)
            agx1_in = dram.tile([128, NJ * QB], F16)
            agx1_out = dram.tile([NC * 128, NJ * QB], F16)

            # ---------------- P0: xW0T quarter + AllGather ----------------
            with tc.tile_pool(name="w0p", bufs=1) as w0p, \
                 tc.tile_pool(name="p0", bufs=3) as p0, \
                 tc.tile_pool(name="p0ps", bufs=4, space="PSUM") as p0ps:
                w0 = w0p.tile([128, K0, G], F16)
                nc.sync.dma_start(w0[:], wih0T.ap())
                b0 = w0p.tile([128, NJ], F32)
                nc.sync.dma_start(b0[:], b0T.ap())
                x0q = w0p.tile([128, NJ, QB], F16, tag="x0q")
                for cc in range(QB // 512):
                    jt = p0.tile([128, K0, 512], F16, tag="jt")
                    nc.sync.dma_start(jt[:], jointT.ap()[:, :, cc * 512:(cc + 1) * 512])
                    for j in range(NJ):
                        ps = p0ps.tile([128, 512], F32, tag="ps")
                        for k in range(K0):
                            nc.tensor.matmul(ps[:], w0[:, k, j * 128:(j + 1) * 128],
                                             jt[:, k, :], start=(k == 0), stop=(k == K0 - 1))
                        nc.vector.tensor_scalar(x0q[:, j, cc * 512:(cc + 1) * 512],
                                                ps[:], b0[:, j:j + 1], None,
                                                mybir.AluOpType.add)
                nc.sync.dma_start(agx0_in[:], x0q[:].rearrange("p j t -> p (j t)"))
                nc.gpsimd.collective_compute(
                    "AllGather", mybir.AluOpType.bypass,
                    replica_groups=[list(range(NC))],
                    ins=[agx0_in.opt()], outs=[agx0_out.opt()])

            md = pers.tile([128, 1], F32, tag="md")
            nc.sync.dma_start(md[:], mdir.ap())

            # ---------------- chain helper ----------------
            def run_chain(whhT_in, agx_out, bhnT_in, y_buf, hid_out):
                agxv = agx_out[:].rearrange("(r p) (j t) -> r p j t", p=128, j=NJ)
                with tc.tile_pool(name="whp", bufs=1) as whp:
                    whh = whp.tile([128, KH, G], F16, tag="whh")
                    nc.sync.dma_start(whh[:], whhT_in.ap())
                    bhn = whp.tile([128, KH], F32, tag="bhn")
                    nc.sync.dma_start(bhn[:], bhnT_in.ap())
                    hzero = whp.tile([128, KH, B], F16, tag="hzero")
                    nc.any.memset(hzero[:], 0.0)
                    with tc.tile_pool(name="chain", bufs=3) as ch, \
                         tc.tile_pool(name="chps", bufs=2, space="PSUM") as chps:
                        for s in range(T):
                            h_prev = (hzero[:] if s == 0
                                      else y_buf[:, :, (s - 1) * B:s * B])
                            # xw[t=chain pos s]: fwd cores use rank s//12,
                            # bwd cores rank 4+s//12; blend by mdir.
                            qa, off = s // 12, (s % 12) * B
                            xa = ch.tile([128, NJ, B], F16, tag="xa")
                            nc.sync.dma_start(xa[:], agxv[qa, :, :, off:off + B])
                            xb = ch.tile([128, NJ, B], F16, tag="xb")
                            nc.sync.dma_start(xb[:], agxv[4 + qa, :, :, off:off + B])
                            xw = ch.tile([128, NJ, B], F32, tag="xw")
                            nc.gpsimd.tensor_tensor(xw[:], xa[:], xb[:],
                                                    mybir.AluOpType.subtract)
                            nc.vector.scalar_tensor_tensor(
                                xw[:], xw[:], md[:, 0:1], xb[:],
                                op0=mybir.AluOpType.mult, op1=mybir.AluOpType.add)
                            ps_rz = chps.tile([128, 8, B], F32, tag="ps_rz")
                            ps_n = chps.tile([128, KH, B], F32, tag="ps_n")
                            for j in range(8):
                                for k in range(KH):
                                    nc.tensor.matmul(ps_rz[:, j, :],
                                                     whh[:, k, j * 128:(j + 1) * 128],
                                                     h_prev[:, k, :],
                                                     start=(k == 0), stop=(k == KH - 1))
                            for j in range(KH):
                                for k in range(KH):
                                    nc.tensor.matmul(ps_n[:, j, :],
                                                     whh[:, k, (8 + j) * 128:(9 + j) * 128],
                                                     h_prev[:, k, :],
                                                     start=(k == 0), stop=(k == KH - 1))
                            rz = ch.tile([128, 8, B], F32, tag="rz")
                            nc.vector.tensor_tensor(rz[:], ps_rz[:], xw[:, 0:8, :],
                                                    mybir.AluOpType.add)
                            rzs = ch.tile([128, 8, B], F32, tag="rzs")
                            nc.scalar.activation(rzs[:], rz[:],
                                                 mybir.ActivationFunctionType.Sigmoid)
                            m = ch.tile([128, KH, B], F32, tag="m")
                            for j in range(KH):
                                nc.vector.scalar_tensor_tensor(
                                    m[:, j, :], ps_n[:, j, :], bhn[:, j:j + 1],
                                    rzs[:, j, :],
                                    op0=mybir.AluOpType.add, op1=mybir.AluOpType.mult)
                            a = ch.tile([128, KH, B], F32, tag="a")
                            nc.vector.tensor_tensor(a[:], m[:], xw[:, 8:12, :],
                                                    mybir.AluOpType.add)
                            n_t = ch.tile([128, KH, B], F32, tag="n_t")
                            nc.scalar.activation(n_t[:], a[:],
                                                 mybir.ActivationFunctionType.Tanh)
                            s1 = ch.tile([128, KH, B], F32, tag="s1")
                            nc.gpsimd.tensor_tensor(s1[:], h_prev, n_t[:],
                                                    mybir.AluOpType.subtract)
                            s2 = ch.tile([128, KH, B], F32, tag="s2")
                            nc.vector.tensor_tensor(s2[:], rzs[:, 4:8, :], s1[:],
                                                    mybir.AluOpType.mult)
                            nc.vector.tensor_tensor(y_buf[:, :, s * B:(s + 1) * B],
                                                    n_t[:], s2[:], mybir.AluOpType.add)
                    if hid_out is not None:
                        nc.sync.dma_start(hid_out.ap(),
                                          y_buf[:, :, (T - 1) * B:T * B])

            # ---------------- Phase A + y0 AllGather ----------------
            with tc.tile_pool(name="yAp", bufs=1) as yAp:
                yA = yAp.tile([128, KH, TB], F16, tag="yA")
                run_chain(whh0T, agx0_out, bhn0T, yA, out_hidA)
                nc.sync.dma_start(ag0_in[:], yA[:].rearrange("p k t -> p (k t)"))
                nc.gpsimd.collective_compute(
                    "AllGather", mybir.AluOpType.bypass,
                    replica_groups=[list(range(NC))],
                    ins=[ag0_in.opt()], outs=[ag0_out.opt()])

                # ---- stage quarter selections (ownq from yA, othq from other dir) ----
                with tc.tile_pool(name="selp", bufs=1) as selp:
                    wqs = selp.tile([128, 4], F32, tag="wqs")
                    nc.sync.dma_start(wqs[:], wq.ap())
                    ownq = selp.tile([128, KH, QB], F16, tag="ownq")
                    othq = selp.tile([128, KH, QB], F16, tag="othq")

                    def sel_quarter(src, dst):
                        for k in range(KH):
                            nc.vector.tensor_scalar(
                                dst[:, k, :], src[:, k, 0:QB], wqs[:, 0:1], None,
                                mybir.AluOpType.mult)
                            for qq in range(1, 4):
                                nc.vector.scalar_tensor_tensor(
                                    dst[:, k, :], src[:, k, qq * QB:(qq + 1) * QB],
                                    wqs[:, qq:qq + 1], dst[:, k, :],
                                    op0=mybir.AluOpType.mult,
                                    op1=mybir.AluOpType.add)

                    sel_quarter(yA, ownq)

                    # other-direction y0, reversed into THIS core's chain order,
                    # blended rank0/rank4 by mdir; scoped so it frees before xW1.
                    with tc.tile_pool(name="othp", bufs=1) as othp, \
                         tc.tile_pool(name="blp", bufs=3) as blp:
                        oth = othp.tile([128, KH, TB], F16, tag="oth")
                        ag0v = ag0_out[:].rearrange("(r p) (k t) -> r p k t",
                                                    p=128, k=KH)
                        for k in range(KH):
                            for qq in range(4):
                                nt = 12 * B
                                fa = blp.tile([128, nt], F16, tag="fa")
                                fb = blp.tile([128, nt], F16, tag="fb")
                                for tt in range(12):
                                    s_ot = T - 1 - (qq * 12 + tt)
                                    nc.sync.dma_start(
                                        fa[:, tt * B:(tt + 1) * B],
                                        ag0v[0, :, k, s_ot * B:(s_ot + 1) * B])
                                    nc.sync.dma_start(
                                        fb[:, tt * B:(tt + 1) * B],
                                        ag0v[4, :, k, s_ot * B:(s_ot + 1) * B])
                                d = blp.tile([128, nt], F32, tag="d")
                                nc.vector.tensor_tensor(d[:], fb[:], fa[:],
                                                        mybir.AluOpType.subtract)
                                # oth = fa + mdir*(fb - fa): fwd -> rank4 (bwd)
                                nc.vector.scalar_tensor_tensor(
                                    oth[:, k, qq * nt:(qq + 1) * nt], d[:],
                                    md[:, 0:1], fa[:],
                                    op0=mybir.AluOpType.mult,
                                    op1=mybir.AluOpType.add)
                        sel_quarter(oth, othq)

                    # ---- xW1T quarter: own half + other half; then AllGather ----
                    with tc.tile_pool(name="w1p", bufs=1) as w1p, \
                         tc.tile_pool(name="x1ps", bufs=4, space="PSUM") as x1ps:
                        w1o = w1p.tile([128, KH, G], F16, tag="w1o")
                        w1x = w1p.tile([128, KH, G], F16, tag="w1x")
                        nc.sync.dma_start(w1o[:], wih1To.ap())
                        nc.sync.dma_start(w1x[:], wih1Tx.ap())
                        b1 = w1p.tile([128, NJ], F32, tag="b1")
                        nc.sync.dma_start(b1[:], b1T.ap())
                        x1q = w1p.tile([128, NJ, QB], F16, tag="x1q")
                        for cc in range(QB // 512):
                            sl = slice(cc * 512, (cc + 1) * 512)
                            for j in range(NJ):
                                js = slice(j * 128, (j + 1) * 128)
                                ps = x1ps.tile([128, 512], F32, tag="x1ps")
                                for k in range(KH):
                                    nc.tensor.matmul(ps[:], w1o[:, k, js],
                                                     ownq[:, k, sl],
                                                     start=(k == 0), stop=False)
                                for k in range(KH):
                                    nc.tensor.matmul(ps[:], w1x[:, k, js],
                                                     othq[:, k, sl],
                                                     start=False, stop=(k == KH - 1))
                                nc.vector.tensor_scalar(x1q[:, j, sl], ps[:],
                                                        b1[:, j:j + 1], None,
                                                        mybir.AluOpType.add)
                        nc.sync.dma_start(agx1_in[:],
                                          x1q[:].rearrange("p j t -> p (j t)"))
                        nc.gpsimd.collective_compute(
                            "AllGather", mybir.AluOpType.bypass,
                            replica_groups=[list(range(NC))],
                            ins=[agx1_in.opt()], outs=[agx1_out.opt()])

            # ---------------- Phase B + y1 AllGather ----------------
            with tc.tile_pool(name="yBp", bufs=1) as yBp:
                yB = yBp.tile([128, KH, TB], F16, tag="yB")
                run_chain(whh1T, agx1_out, bhn1T, yB, out_hidB)
                nc.sync.dma_start(ag1_in[:], yB[:].rearrange("p k t -> p (k t)"))
                nc.gpsimd.collective_compute(
                    "AllGather", mybir.AluOpType.bypass,
                    replica_groups=[list(range(NC))],
                    ins=[ag1_in.opt()], outs=[ag1_out.opt()])

            # ---------------- outputs assembly (canonical t) ----------------
            ag1v = ag1_out[:].rearrange("(r p) (k t) -> r p k t", p=128, k=KH)
            with tc.tile_pool(name="oasm", bufs=3) as oasm, \
                 tc.tile_pool(name="o16p", bufs=1) as o16p:
                o16 = o16p.tile([128, KH, TB], F16, tag="o16")
                for cc in range(TB // 512):
                    sl = slice(cc * 512, (cc + 1) * 512)
                    fsb = oasm.tile([128, KH, 512], F16, tag="fsb")
                    nc.sync.dma_start(fsb[:], ag1v[0, :, :, sl])  # fwd: pos == t
                    bsb = oasm.tile([128, KH, 512], F16, tag="bsb")
                    for tt in range(4):
                        t_can = cc * 4 + tt
                        pos = T - 1 - t_can
                        nc.sync.dma_start(bsb[:, :, tt * B:(tt + 1) * B],
                                          ag1v[4, :, :, pos * B:(pos + 1) * B])
                    nc.vector.tensor_tensor(o16[:, :, sl], fsb[:], bsb[:],
                                            mybir.AluOpType.add)
                    nc.sync.dma_start(out_oT.ap()[:, :, sl], o16[:, :, sl])

                # ---------------- classifier ----------------
                with tc.tile_pool(name="cwp", bufs=1) as cwp, \
                     tc.tile_pool(name="clp", bufs=3) as clp, \
                     tc.tile_pool(name="clps", bufs=4, space="PSUM") as clps:
                    cw1 = cwp.tile([128, KH, H], F16, tag="cw1")
                    nc.sync.dma_start(cw1[:], cw1T.ap())
                    cb1 = cwp.tile([128, KH], F32, tag="cb1")
                    nc.sync.dma_start(cb1[:], cb1T.ap())
                    h1 = cwp.tile([128, KH, TB], F16, tag="h1")
                    for cc in range(TB // 512):
                        sl = slice(cc * 512, (cc + 1) * 512)
                        for j in range(KH):
                            ps = clps.tile([128, 512], F32, tag="h1ps")
                            for k in range(KH):
                                nc.tensor.matmul(ps[:], cw1[:, k, j * 128:(j + 1) * 128],
                                                 o16[:, k, sl],
                                                 start=(k == 0), stop=(k == KH - 1))
                            nc.vector.tensor_scalar(h1[:, j, sl], ps[:],
                                                    cb1[:, j:j + 1], 0.0,
                                                    mybir.AluOpType.add,
                                                    mybir.AluOpType.max)
                    cw2 = cwp.tile([128, KH, VS], F16, tag="cw2")
                    nc.sync.dma_start(cw2[:], cw2T.ap())
                    cb2 = cwp.tile([128, VS], F32, tag="cb2")
                    nc.sync.dma_start(cb2[:], cb2b.ap())
                    ocls = out_cls.ap()  # [T, B, VS]
                    for m in range(T):
                        ms = slice(m * 128, (m + 1) * 128)
                        for v0 in range(0, VS, 512):
                            vn = min(512, VS - v0)
                            ps = clps.tile([128, 512], F32, tag="clsps")
                            for k in range(KH):
                                nc.tensor.matmul(ps[:, :vn], h1[:, k, ms],
                                                 cw2[:, k, v0:v0 + vn],
                                                 start=(k == 0), stop=(k == KH - 1))
                            o = clp.tile([128, 512], F16, tag="clso")
                            nc.vector.tensor_tensor(o[:, :vn], ps[:, :vn],
                                                    cb2[:, v0:v0 + vn],
                                                    mybir.AluOpType.add)
                            nc.sync.dma_start(ocls[m, :, v0:v0 + vn], o[:, :vn])

    nc.finalize()
    return nc


def _tp(w):
    """[out, in] -> [128, in//128, out] fp16 (K-on-partition transposed layout)."""
    o, i = w.shape
    return np.ascontiguousarray(
        w.T.reshape(i // 128, 128, o).transpose(1, 0, 2)).astype(np.float16)


def _tvec(v):
    """[n] -> [128, n//128] (feature-on-partition)."""
    n = v.shape[0]
    return np.ascontiguousarray(v.reshape(n // 128, 128).T).astype(np.float32)


def kernel(input_seqs, char2num, char2final, emb_w, sound_w, num_w,
           gru_params, cW1, cb1, cW2, cb2):
    input_seqs = np.asarray(input_seqs)
    char2num = np.asarray(char2num)
    char2final = np.asarray(char2final)
    emb_w = np.asarray(emb_w, np.float32)
    sound_w = np.asarray(sound_w, np.float32)
    num_w = np.asarray(num_w, np.float32)
    cW1 = np.asarray(cW1, np.float32)
    cb1 = np.asarray(cb1, np.float32)
    cW2 = np.asarray(cW2, np.float32)
    cb2 = np.asarray(cb2, np.float32)

    if "nc" not in _CACHE:
        _CACHE["nc"] = _build_program()
    nc = _CACHE["nc"]

    # ---- host-side input prep ----
    ids = input_seqs.astype(np.int64)
    joint = np.zeros((T, B, K0 * 128), np.float32)
    joint[:, :, :H] = emb_w[ids]
    joint[:, :, H:H + ED] = sound_w[char2final[ids]]
    joint[:, :, H + ED:IN0] = num_w[char2num[ids]]

    gp = gru_params
    if "wmaps" not in _CACHE:
        cw1T_a, cb1T_a, cw2T_a = _tp(cW1), _tvec(cb1), _tp(cW2)
        wmaps = []
        for c in range(NC):
            d = c // 4  # 0 fwd, 1 bwd
            q = c % 4
            wq_v = np.zeros((128, 4), np.float32)
            wq_v[:, q] = 1.0
            p0 = gp[0][d]
            p1 = gp[1][d]
            wih0 = np.zeros((G, K0 * 128), np.float32)
            wih0[:, :IN0] = np.asarray(p0['Wih'], np.float32)
            wih1 = np.asarray(p1['Wih'], np.float32)       # [G, 1024]
            own, oth = ((slice(0, H), slice(H, 2 * H)) if d == 0
                        else (slice(H, 2 * H), slice(0, H)))
            b0 = np.asarray(p0['bih'], np.float32).copy()
            bhh0 = np.asarray(p0['bhh'], np.float32)
            b0[:2 * H] += bhh0[:2 * H]
            b1v = np.asarray(p1['bih'], np.float32).copy()
            bhh1 = np.asarray(p1['bhh'], np.float32)
            b1v[:2 * H] += bhh1[:2 * H]
            wmaps.append({
                "wih0T": _tp(wih0),
                "whh0T": _tp(np.asarray(p0['Whh'], np.float32)),
                "wih1To": _tp(wih1[:, own]),
                "wih1Tx": _tp(wih1[:, oth]),
                "whh1T": _tp(np.asarray(p1['Whh'], np.float32)),
                "b0T": _tvec(b0),
                "b1T": _tvec(b1v),
                "bhn0T": _tvec(bhh0[2 * H:]),
                "bhn1T": _tvec(bhh1[2 * H:]),
                "cw1T": cw1T_a,
                "cb1T": cb1T_a,
                "cw2T": np.ascontiguousarray(cw2T_a[:, :, c * VS:(c + 1) * VS]),
                "cb2b": np.broadcast_to(cb2[c * VS:(c + 1) * VS],
                                        (128, VS)).astype(np.float32).copy(),
                "mdir": np.full((128, 1), 1.0 - d, np.float32),
                "wq": wq_v,
            })
        _CACHE["wmaps"] = wmaps

    in_maps = []
    for c in range(NC):
        d, q = c // 4, c % 4
        order = np.arange(T) if d == 0 else np.arange(T - 1, -1, -1)
        jc = joint[order][12 * q:12 * (q + 1)]             # [12,B,640] chain quarter
        jT = np.ascontiguousarray(
            jc.reshape(12 * B, K0, 128).transpose(2, 1, 0)).astype(np.float16)
        in_maps.append({"jointT": jT, **_CACHE["wmaps"][c]})

    res = run_bass_kernel_spmd(nc, in_maps, core_ids=list(range(NC)))
    r = res.results

    # ---- host-side reassembly ----
    # outputs: out_oT [128,KH,TB] fp32, canonical t: outputs[t,b,h]
    oT = r[0]["out_oT"].astype(np.float32)                 # [128, 4, 6144]
    outputs = oT.reshape(128, KH, T, B).transpose(2, 3, 1, 0).reshape(T, B, H)
    outputs = np.ascontiguousarray(outputs, np.float32)

    def _hid(x):  # [128, KH, B] fp16 -> [B, H] fp32
        return np.ascontiguousarray(
            x.astype(np.float32).reshape(128, KH, B).transpose(2, 1, 0).reshape(B, H))

    hidden = np.stack([
        _hid(r[0]["out_hidA"]), _hid(r[4]["out_hidA"]),
        _hid(r[0]["out_hidB"]), _hid(r[4]["out_hidB"]),
    ], 0).astype(np.float32)

    cls = np.concatenate([r[c]["out_cls"] for c in range(NC)], axis=2)
    cls = np.ascontiguousarray(cls, np.float32)
    return outputs, hidden, cls


# revision 27
# speedup vs baseline: 1.3334x; 1.3334x over previous
"""Trainium2 Bass kernel for nn_EncoderRNN: 2-layer bidirectional GRU encoder.

Design (8 NeuronCores, one SPMD NEFF):
 - Feature-on-partition ("transposed") layout on device. Host does the cheap
   index gathers, per-core input layout transforms, and final reassembly.
 - Cores 0-3 run forward chains, 4-7 backward chains; all direction asymmetry
   (time order, weight halves, blend masks) is encoded in per-core input DATA
   so a single instruction stream serves all cores.
 - Per layer: xW = x @ Wih.T precomputed as a fat fp16 matmul spilled to local
   DRAM and streamed back per step; the recurrence streams Whh.T 128x128
   stationary tiles against the live h state (fp16 operands, fp32 PSUM).
 - One 8-rank AllGather per layer exchanges full y trajectories.
 - Classifier is vocab-sharded (1000 rows/core) over the [8000,512] projection.
"""
import sys
import numpy as np

try:
    import concourse.bass  # noqa: F401  (already on path in some environments)
except ImportError:
    sys.path.insert(0, '/opt/trn_rl_repo')

import concourse.bass as bass
import concourse.mybir as mybir
import concourse.tile as tile
from concourse import bacc
from concourse.bass_utils import run_bass_kernel_spmd

T, B, H, V, ED = 48, 128, 512, 8000, 32
IN0, K0 = 576, 5            # layer0 input width, padded to 640 = 5*128
G, NJ, KH = 1536, 12, 4
NC = 8
VS = V // NC                # 1000
TB = T * B                  # 6144
F16 = mybir.dt.float16
F32 = mybir.dt.float32

_CACHE = {}


def _build_program():
    nc = bacc.Bacc("TRN2", target_bir_lowering=False, debug=False,
                   enable_asserts=False, num_devices=NC)
    dt = nc.dram_tensor
    QB = 12 * B  # 1536 columns: one chain-quarter of positions
    jointT = dt("jointT", [128, K0, QB], F16, kind="ExternalInput")
    wih0T = dt("wih0T", [128, K0, G], F16, kind="ExternalInput")
    whh0T = dt("whh0T", [128, KH, G], F16, kind="ExternalInput")
    wih1To = dt("wih1To", [128, KH, G], F16, kind="ExternalInput")
    wih1Tx = dt("wih1Tx", [128, KH, G], F16, kind="ExternalInput")
    whh1T = dt("whh1T", [128, KH, G], F16, kind="ExternalInput")
    b0T = dt("b0T", [128, NJ], F32, kind="ExternalInput")
    b1T = dt("b1T", [128, NJ], F32, kind="ExternalInput")
    bhn0T = dt("bhn0T", [128, KH], F32, kind="ExternalInput")
    bhn1T = dt("bhn1T", [128, KH], F32, kind="ExternalInput")
    cw1T = dt("cw1T", [128, KH, H], F16, kind="ExternalInput")
    cb1T = dt("cb1T", [128, KH], F32, kind="ExternalInput")
    cw2T = dt("cw2T", [128, KH, VS], F16, kind="ExternalInput")
    cb2b = dt("cb2b", [128, VS], F32, kind="ExternalInput")
    mdir = dt("mdir", [128, 1], F32, kind="ExternalInput")   # 1.0 on fwd cores, 0.0 on bwd
    wq = dt("wq", [128, 4], F32, kind="ExternalInput")       # one-hot quarter select

    out_cls = dt("out_cls", [T, B, VS], F16, kind="ExternalOutput")
    out_oT = dt("out_oT", [128, KH, TB], F16, kind="ExternalOutput")
    out_hidA = dt("out_hidA", [128, KH, B], F16, kind="ExternalOutput")
    out_hidB = dt("out_hidB", [128, KH, B], F16, kind="ExternalOutput")

    with tile.TileContext(nc) as tc:
        with tc.tile_pool(name="pers", bufs=1) as pers, \
             tc.tile_pool(name="dram", bufs=1, space="DRAM") as dram:

            QB = 12 * B
            ag0_in = dram.tile([128, KH * TB], F16)
            ag0_out = dram.tile([NC * 128, KH * TB], F16)
            ag1_in = dram.tile([128, KH * TB], F16)
            ag1_out = dram.tile([NC * 128, KH * TB], F16)
            agx0_in = dram.tile([128, NJ * QB], F16)
            agx0_out = dram.tile([NC * 128, NJ * QB], F16)
            agx1_in = dram.tile([128, NJ * QB], F16)
            agx1_out = dram.tile([NC * 128, NJ * QB], F16)

            # ---------------- P0: xW0T quarter + AllGather ----------------
            with tc.tile_pool(name="w0p", bufs=1) as w0p, \
                 tc.tile_pool(name="p0", bufs=3) as p0, \
                 tc.tile_pool(name="p0ps", bufs=4, space="PSUM") as p0ps:
                w0 = w0p.tile([128, K0, G], F16)
                nc.sync.dma_start(w0[:], wih0T.ap())
                b0 = w0p.tile([128, NJ], F32)
                nc.sync.dma_start(b0[:], b0T.ap())
                x0q = w0p.tile([128, NJ, QB], F16, tag="x0q")
                for cc in range(QB // 512):
                    jt = p0.tile([128, K0, 512], F16, tag="jt")
                    nc.sync.dma_start(jt[:], jointT.ap()[:, :, cc * 512:(cc + 1) * 512])
                    for j in range(NJ):
                        ps = p0ps.tile([128, 512], F32, tag="ps")
                        for k in range(K0):
                            nc.tensor.matmul(ps[:], w0[:, k, j * 128:(j + 1) * 128],
                                             jt[:, k, :], start=(k == 0), stop=(k == K0 - 1))
                        nc.vector.tensor_scalar(x0q[:, j, cc * 512:(cc + 1) * 512],
                                                ps[:], b0[:, j:j + 1], None,
                                                mybir.AluOpType.add)
                nc.sync.dma_start(agx0_in[:], x0q[:].rearrange("p j t -> p (j t)"))
                nc.gpsimd.collective_compute(
                    "AllGather", mybir.AluOpType.bypass,
                    replica_groups=[list(range(NC))],
                    ins=[agx0_in.opt()], outs=[agx0_out.opt()])

            md = pers.tile([128, 1], F32, tag="md")
            nc.sync.dma_start(md[:], mdir.ap())

            # ---------------- chain helper ----------------
            def run_chain(whhT_in, agx_out, bhnR_in, y_buf, hid_out):
                agxv = agx_out[:].rearrange("(r p) (j t) -> r p j t", p=128, j=NJ)
                with tc.tile_pool(name="whp", bufs=1) as whp:
                    whh = whp.tile([128, KH, G], F16, tag="whh")
                    nc.sync.dma_start(whh[:], whhT_in.ap())
                    bhn = whp.tile([128, KH], F32, tag="bhn")
                    nc.sync.dma_start(bhn[:], bhnR_in.ap())
                    hzero = whp.tile([128, KH, B], F16, tag="hzero")
                    nc.any.memset(hzero[:], 0.0)
                    with tc.tile_pool(name="chain", bufs=3) as ch, \
                         tc.tile_pool(name="chps", bufs=2, space="PSUM") as chps:
                        for s in range(T):
                            h_prev = (hzero[:] if s == 0
                                      else y_buf[:, :, (s - 1) * B:s * B])
                            # xw[t=chain pos s]: fwd cores use rank s//12,
                            # bwd cores rank 4+s//12; blend by mdir.
                            qa, off = s // 12, (s % 12) * B
                            xa = ch.tile([128, NJ, B], F16, tag="xa")
                            nc.sync.dma_start(xa[:], agxv[qa, :, :, off:off + B])
                            xb = ch.tile([128, NJ, B], F16, tag="xb")
                            nc.sync.dma_start(xb[:], agxv[4 + qa, :, :, off:off + B])
                            xw = ch.tile([128, NJ, B], F32, tag="xw")
                            nc.gpsimd.tensor_tensor(xw[:], xa[:], xb[:],
                                                    mybir.AluOpType.subtract)
                            nc.vector.scalar_tensor_tensor(
                                xw[:], xw[:], md[:, 0:1], xb[:],
                                op0=mybir.AluOpType.mult, op1=mybir.AluOpType.add)
                            ps_rz = chps.tile([128, 8, B], F32, tag="ps_rz")
                            ps_n = chps.tile([128, KH, B], F32, tag="ps_n")
                            for j in range(8):
                                for k in range(KH):
                                    nc.tensor.matmul(ps_rz[:, j, :],
                                                     whh[:, k, j * 128:(j + 1) * 128],
                                                     h_prev[:, k, :],
                                                     start=(k == 0), stop=(k == KH - 1))
                            for j in range(KH):
                                for k in range(KH):
                                    nc.tensor.matmul(ps_n[:, j, :],
                                                     whh[:, k, (8 + j) * 128:(9 + j) * 128],
                                                     h_prev[:, k, :],
                                                     start=(k == 0), stop=(k == KH - 1))
                            rz = ch.tile([128, 8, B], F32, tag="rz")
                            nc.vector.tensor_tensor(rz[:], ps_rz[:], xw[:, 0:8, :],
                                                    mybir.AluOpType.add)
                            rzs = ch.tile([128, 8, B], F32, tag="rzs")
                            nc.scalar.activation(rzs[:], rz[:],
                                                 mybir.ActivationFunctionType.Sigmoid)
                            # off-critical-path: w = z*h_prev, u = 1-z (overlap tanh)
                            w_t = ch.tile([128, KH, B], F32, tag="w_t")
                            nc.vector.tensor_tensor(w_t[:], rzs[:, 4:8, :], h_prev,
                                                    mybir.AluOpType.mult)
                            u_t = ch.tile([128, KH, B], F32, tag="u_t")
                            nc.vector.tensor_scalar(u_t[:], rzs[:, 4:8, :], -1.0, 1.0,
                                                    mybir.AluOpType.mult,
                                                    mybir.AluOpType.add)
                            m = ch.tile([128, KH, B], F32, tag="m")
                            for j in range(KH):
                                nc.vector.scalar_tensor_tensor(
                                    m[:, j, :], ps_n[:, j, :], bhn[:, j:j + 1],
                                    rzs[:, j, :],
                                    op0=mybir.AluOpType.add, op1=mybir.AluOpType.mult)
                            a = ch.tile([128, KH, B], F32, tag="a")
                            nc.vector.tensor_tensor(a[:], m[:], xw[:, 8:12, :],
                                                    mybir.AluOpType.add)
                            n_t = ch.tile([128, KH, B], F32, tag="n_t")
                            nc.scalar.activation(n_t[:], a[:],
                                                 mybir.ActivationFunctionType.Tanh)
                            s2 = ch.tile([128, KH, B], F32, tag="s2")
                            nc.vector.tensor_tensor(s2[:], n_t[:], u_t[:],
                                                    mybir.AluOpType.mult)
                            nc.vector.tensor_tensor(y_buf[:, :, s * B:(s + 1) * B],
                                                    s2[:], w_t[:], mybir.AluOpType.add)
                    if hid_out is not None:
                        nc.sync.dma_start(hid_out.ap(),
                                          y_buf[:, :, (T - 1) * B:T * B])

            # ---------------- Phase A + y0 AllGather ----------------
            with tc.tile_pool(name="yAp", bufs=1) as yAp:
                yA = yAp.tile([128, KH, TB], F16, tag="yA")
                run_chain(whh0T, agx0_out, bhn0T, yA, out_hidA)
                nc.sync.dma_start(ag0_in[:], yA[:].rearrange("p k t -> p (k t)"))
                nc.gpsimd.collective_compute(
                    "AllGather", mybir.AluOpType.bypass,
                    replica_groups=[list(range(NC))],
                    ins=[ag0_in.opt()], outs=[ag0_out.opt()])

                # ---- stage quarter selections (ownq from yA, othq from other dir) ----
                with tc.tile_pool(name="selp", bufs=1) as selp:
                    wqs = selp.tile([128, 4], F32, tag="wqs")
                    nc.sync.dma_start(wqs[:], wq.ap())
                    ownq = selp.tile([128, KH, QB], F16, tag="ownq")
                    othq = selp.tile([128, KH, QB], F16, tag="othq")

                    def sel_quarter(src, dst):
                        for k in range(KH):
                            nc.vector.tensor_scalar(
                                dst[:, k, :], src[:, k, 0:QB], wqs[:, 0:1], None,
                                mybir.AluOpType.mult)
                            for qq in range(1, 4):
                                nc.vector.scalar_tensor_tensor(
                                    dst[:, k, :], src[:, k, qq * QB:(qq + 1) * QB],
                                    wqs[:, qq:qq + 1], dst[:, k, :],
                                    op0=mybir.AluOpType.mult,
                                    op1=mybir.AluOpType.add)

                    sel_quarter(yA, ownq)

                    # other-direction y0, reversed into THIS core's chain order,
                    # blended rank0/rank4 by mdir; scoped so it frees before xW1.
                    with tc.tile_pool(name="othp", bufs=1) as othp, \
                         tc.tile_pool(name="blp", bufs=3) as blp:
                        oth = othp.tile([128, KH, TB], F16, tag="oth")
                        ag0v = ag0_out[:].rearrange("(r p) (k t) -> r p k t",
                                                    p=128, k=KH)
                        for k in range(KH):
                            for qq in range(4):
                                nt = 12 * B
                                fa = blp.tile([128, nt], F16, tag="fa")
                                fb = blp.tile([128, nt], F16, tag="fb")
                                for tt in range(12):
                                    s_ot = T - 1 - (qq * 12 + tt)
                                    nc.sync.dma_start(
                                        fa[:, tt * B:(tt + 1) * B],
                                        ag0v[0, :, k, s_ot * B:(s_ot + 1) * B])
                                    nc.sync.dma_start(
                                        fb[:, tt * B:(tt + 1) * B],
                                        ag0v[4, :, k, s_ot * B:(s_ot + 1) * B])
                                d = blp.tile([128, nt], F32, tag="d")
                                nc.vector.tensor_tensor(d[:], fb[:], fa[:],
                                                        mybir.AluOpType.subtract)
                                # oth = fa + mdir*(fb - fa): fwd -> rank4 (bwd)
                                nc.vector.scalar_tensor_tensor(
                                    oth[:, k, qq * nt:(qq + 1) * nt], d[:],
                                    md[:, 0:1], fa[:],
                                    op0=mybir.AluOpType.mult,
                                    op1=mybir.AluOpType.add)
                        sel_quarter(oth, othq)

                    # ---- xW1T quarter: own half + other half; then AllGather ----
                    with tc.tile_pool(name="w1p", bufs=1) as w1p, \
                         tc.tile_pool(name="x1ps", bufs=4, space="PSUM") as x1ps:
                        w1o = w1p.tile([128, KH, G], F16, tag="w1o")
                        w1x = w1p.tile([128, KH, G], F16, tag="w1x")
                        nc.sync.dma_start(w1o[:], wih1To.ap())
                        nc.sync.dma_start(w1x[:], wih1Tx.ap())
                        b1 = w1p.tile([128, NJ], F32, tag="b1")
                        nc.sync.dma_start(b1[:], b1T.ap())
                        x1q = w1p.tile([128, NJ, QB], F16, tag="x1q")
                        for cc in range(QB // 512):
                            sl = slice(cc * 512, (cc + 1) * 512)
                            for j in range(NJ):
                                js = slice(j * 128, (j + 1) * 128)
                                ps = x1ps.tile([128, 512], F32, tag="x1ps")
                                for k in range(KH):
                                    nc.tensor.matmul(ps[:], w1o[:, k, js],
                                                     ownq[:, k, sl],
                                                     start=(k == 0), stop=False)
                                for k in range(KH):
                                    nc.tensor.matmul(ps[:], w1x[:, k, js],
                                                     othq[:, k, sl],
                                                     start=False, stop=(k == KH - 1))
                                nc.vector.tensor_scalar(x1q[:, j, sl], ps[:],
                                                        b1[:, j:j + 1], None,
                                                        mybir.AluOpType.add)
                        nc.sync.dma_start(agx1_in[:],
                                          x1q[:].rearrange("p j t -> p (j t)"))
                        nc.gpsimd.collective_compute(
                            "AllGather", mybir.AluOpType.bypass,
                            replica_groups=[list(range(NC))],
                            ins=[agx1_in.opt()], outs=[agx1_out.opt()])

            # ---------------- Phase B + y1 AllGather ----------------
            with tc.tile_pool(name="yBp", bufs=1) as yBp:
                yB = yBp.tile([128, KH, TB], F16, tag="yB")
                run_chain(whh1T, agx1_out, bhn1T, yB, out_hidB)
                nc.sync.dma_start(ag1_in[:], yB[:].rearrange("p k t -> p (k t)"))
                nc.gpsimd.collective_compute(
                    "AllGather", mybir.AluOpType.bypass,
                    replica_groups=[list(range(NC))],
                    ins=[ag1_in.opt()], outs=[ag1_out.opt()])

            # ---------------- outputs assembly (canonical t) ----------------
            ag1v = ag1_out[:].rearrange("(r p) (k t) -> r p k t", p=128, k=KH)
            with tc.tile_pool(name="oasm", bufs=3) as oasm, \
                 tc.tile_pool(name="o16p", bufs=1) as o16p:
                o16 = o16p.tile([128, KH, TB], F16, tag="o16")
                for cc in range(TB // 512):
                    sl = slice(cc * 512, (cc + 1) * 512)
                    fsb = oasm.tile([128, KH, 512], F16, tag="fsb")
                    nc.sync.dma_start(fsb[:], ag1v[0, :, :, sl])  # fwd: pos == t
                    bsb = oasm.tile([128, KH, 512], F16, tag="bsb")
                    for tt in range(4):
                        t_can = cc * 4 + tt
                        pos = T - 1 - t_can
                        nc.sync.dma_start(bsb[:, :, tt * B:(tt + 1) * B],
                                          ag1v[4, :, :, pos * B:(pos + 1) * B])
                    nc.vector.tensor_tensor(o16[:, :, sl], fsb[:], bsb[:],
                                            mybir.AluOpType.add)
                    nc.sync.dma_start(out_oT.ap()[:, :, sl], o16[:, :, sl])

                # ---------------- classifier ----------------
                with tc.tile_pool(name="cwp", bufs=1) as cwp, \
                     tc.tile_pool(name="clp", bufs=3) as clp, \
                     tc.tile_pool(name="clps", bufs=4, space="PSUM") as clps:
                    cw1 = cwp.tile([128, KH, H], F16, tag="cw1")
                    nc.sync.dma_start(cw1[:], cw1T.ap())
                    cb1 = cwp.tile([128, KH], F32, tag="cb1")
                    nc.sync.dma_start(cb1[:], cb1T.ap())
                    h1 = cwp.tile([128, KH, TB], F16, tag="h1")
                    for cc in range(TB // 512):
                        sl = slice(cc * 512, (cc + 1) * 512)
                        for j in range(KH):
                            ps = clps.tile([128, 512], F32, tag="h1ps")
                            for k in range(KH):
                                nc.tensor.matmul(ps[:], cw1[:, k, j * 128:(j + 1) * 128],
                                                 o16[:, k, sl],
                                                 start=(k == 0), stop=(k == KH - 1))
                            nc.vector.tensor_scalar(h1[:, j, sl], ps[:],
                                                    cb1[:, j:j + 1], 0.0,
                                                    mybir.AluOpType.add,
                                                    mybir.AluOpType.max)
                    cw2 = cwp.tile([128, KH, VS], F16, tag="cw2")
                    nc.sync.dma_start(cw2[:], cw2T.ap())
                    cb2 = cwp.tile([128, VS], F32, tag="cb2")
                    nc.sync.dma_start(cb2[:], cb2b.ap())
                    ocls = out_cls.ap()  # [T, B, VS]
                    for m in range(T):
                        ms = slice(m * 128, (m + 1) * 128)
                        for v0 in range(0, VS, 512):
                            vn = min(512, VS - v0)
                            ps = clps.tile([128, 512], F32, tag="clsps")
                            for k in range(KH):
                                nc.tensor.matmul(ps[:, :vn], h1[:, k, ms],
                                                 cw2[:, k, v0:v0 + vn],
                                                 start=(k == 0), stop=(k == KH - 1))
                            o = clp.tile([128, 512], F16, tag="clso")
                            nc.vector.tensor_tensor(o[:, :vn], ps[:, :vn],
                                                    cb2[:, v0:v0 + vn],
                                                    mybir.AluOpType.add)
                            nc.sync.dma_start(ocls[m, :, v0:v0 + vn], o[:, :vn])

    nc.finalize()
    return nc


def _tp(w):
    """[out, in] -> [128, in//128, out] fp16 (K-on-partition transposed layout)."""
    o, i = w.shape
    return np.ascontiguousarray(
        w.T.reshape(i // 128, 128, o).transpose(1, 0, 2)).astype(np.float16)


def _tvec(v):
    """[n] -> [128, n//128] (feature-on-partition)."""
    n = v.shape[0]
    return np.ascontiguousarray(v.reshape(n // 128, 128).T).astype(np.float32)


def kernel(input_seqs, char2num, char2final, emb_w, sound_w, num_w,
           gru_params, cW1, cb1, cW2, cb2):
    input_seqs = np.asarray(input_seqs)
    char2num = np.asarray(char2num)
    char2final = np.asarray(char2final)
    emb_w = np.asarray(emb_w, np.float32)
    sound_w = np.asarray(sound_w, np.float32)
    num_w = np.asarray(num_w, np.float32)
    cW1 = np.asarray(cW1, np.float32)
    cb1 = np.asarray(cb1, np.float32)
    cW2 = np.asarray(cW2, np.float32)
    cb2 = np.asarray(cb2, np.float32)

    if "nc" not in _CACHE:
        _CACHE["nc"] = _build_program()
    nc = _CACHE["nc"]

    # ---- host-side input prep ----
    ids = input_seqs.astype(np.int64)
    joint = np.zeros((T, B, K0 * 128), np.float32)
    joint[:, :, :H] = emb_w[ids]
    joint[:, :, H:H + ED] = sound_w[char2final[ids]]
    joint[:, :, H + ED:IN0] = num_w[char2num[ids]]

    gp = gru_params
    if "wmaps" not in _CACHE:
        cw1T_a, cb1T_a, cw2T_a = _tp(cW1), _tvec(cb1), _tp(cW2)
        wmaps = []
        for c in range(NC):
            d = c // 4  # 0 fwd, 1 bwd
            q = c % 4
            wq_v = np.zeros((128, 4), np.float32)
            wq_v[:, q] = 1.0
            p0 = gp[0][d]
            p1 = gp[1][d]
            wih0 = np.zeros((G, K0 * 128), np.float32)
            wih0[:, :IN0] = np.asarray(p0['Wih'], np.float32)
            wih1 = np.asarray(p1['Wih'], np.float32)       # [G, 1024]
            own, oth = ((slice(0, H), slice(H, 2 * H)) if d == 0
                        else (slice(H, 2 * H), slice(0, H)))
            b0 = np.asarray(p0['bih'], np.float32).copy()
            bhh0 = np.asarray(p0['bhh'], np.float32)
            b0[:2 * H] += bhh0[:2 * H]
            b1v = np.asarray(p1['bih'], np.float32).copy()
            bhh1 = np.asarray(p1['bhh'], np.float32)
            b1v[:2 * H] += bhh1[:2 * H]
            wmaps.append({
                "wih0T": _tp(wih0),
                "whh0T": _tp(np.asarray(p0['Whh'], np.float32)),
                "wih1To": _tp(wih1[:, own]),
                "wih1Tx": _tp(wih1[:, oth]),
                "whh1T": _tp(np.asarray(p1['Whh'], np.float32)),
                "b0T": _tvec(b0),
                "b1T": _tvec(b1v),
                "bhn0T": _tvec(bhh0[2 * H:]),
                "bhn1T": _tvec(bhh1[2 * H:]),
                "cw1T": cw1T_a,
                "cb1T": cb1T_a,
                "cw2T": np.ascontiguousarray(cw2T_a[:, :, c * VS:(c + 1) * VS]),
                "cb2b": np.broadcast_to(cb2[c * VS:(c + 1) * VS],
                                        (128, VS)).astype(np.float32).copy(),
                "mdir": np.full((128, 1), 1.0 - d, np.float32),
                "wq": wq_v,
            })
        _CACHE["wmaps"] = wmaps

    in_maps = []
    for c in range(NC):
        d, q = c // 4, c % 4
        order = np.arange(T) if d == 0 else np.arange(T - 1, -1, -1)
        jc = joint[order][12 * q:12 * (q + 1)]             # [12,B,640] chain quarter
        jT = np.ascontiguousarray(
            jc.reshape(12 * B, K0, 128).transpose(2, 1, 0)).astype(np.float16)
        in_maps.append({"jointT": jT, **_CACHE["wmaps"][c]})

    res = run_bass_kernel_spmd(nc, in_maps, core_ids=list(range(NC)))
    r = res.results

    # ---- host-side reassembly ----
    # outputs: out_oT [128,KH,TB] fp32, canonical t: outputs[t,b,h]
    oT = r[0]["out_oT"].astype(np.float32)                 # [128, 4, 6144]
    outputs = oT.reshape(128, KH, T, B).transpose(2, 3, 1, 0).reshape(T, B, H)
    outputs = np.ascontiguousarray(outputs, np.float32)

    def _hid(x):  # [128, KH, B] fp16 -> [B, H] fp32
        return np.ascontiguousarray(
            x.astype(np.float32).reshape(128, KH, B).transpose(2, 1, 0).reshape(B, H))

    hidden = np.stack([
        _hid(r[0]["out_hidA"]), _hid(r[4]["out_hidA"]),
        _hid(r[0]["out_hidB"]), _hid(r[4]["out_hidB"]),
    ], 0).astype(np.float32)

    cls = np.concatenate([r[c]["out_cls"] for c in range(NC)], axis=2)
    cls = np.ascontiguousarray(cls, np.float32)
    return outputs, hidden, cls
